# revision 11
# baseline (speedup 1.0000x reference)
"""Trainium2 Bass kernel for MetaGIN-style GNN message passing.

Strategy (edge-sharded, tgt-sorted):
  * Sort edges by target node. Nodes are grouped into 128-node blocks;
    blocks are assigned contiguously to the 8 cores (10 block slots each).
  * Phase A (replicated on each core): xs|xt = x @ [W_src.T | W_tgt.T]
    computed on-device into an internal DRAM table [NPAD, 512].
  * Phase B: per node block, gather xs[src]/xt[tgt] rows with dma_gather,
    GroupNorm via bn_stats, gated MLP via block-diagonal matmuls (the
    EmbeddingBag term enters as a host-precomputed per-edge histogram
    matmul'd against emb @ BDg.T, accumulated in the same PSUM group),
    then scatter-add to nodes with a one-hot matmul (ST.T @ act) that
    accumulates in PSUM across all edge tiles of the block.
  * Per block: agg @ W_post.T, then deg^deg_param scaling, write out.
  No collectives needed: each core owns a disjoint contiguous node range.
"""

import math

import numpy as np

import concourse.bass as bass
import concourse.mybir as mybir
import concourse.tile as tile
from concourse import bacc
from concourse.bass_utils import run_bass_kernel_spmd
from concourse.masks import make_identity

P = 128
N = 10000
W = 256
H = 8
D = 32
BOND = 33
E = 320000
EPS_GN = 1e-5
CORES = 8
NPAD = 10240          # 80 * 128
NSLOTS = NPAD // P    # 80 block slots
BPC = NSLOTS // CORES  # 10 block slots per core
CL = 8                # tiles (of 128 edges) per gather chunk
FP = mybir.dt.float32
I16 = mybir.dt.int16
DEBUG = False  # adds intermediate-dump outputs for core 0 / block 0 / tile 0


def _prep_host(x, deg, edge_idx, edge_attr, W_src, W_tgt, emb, W_gate, W_val,
               W_post, deg_param):
    src = np.ascontiguousarray(edge_idx[0]).astype(np.int64)
    tgt = np.ascontiguousarray(edge_idx[1]).astype(np.int64)
    attr = np.asarray(edge_attr).astype(np.int64)

    order = np.argsort(tgt, kind="stable")
    s_src = src[order]
    s_tgt = tgt[order]
    s_attr = attr[order]

    blk = (s_tgt // P).astype(np.int64)
    counts = np.bincount(blk, minlength=NSLOTS)
    starts = np.concatenate([[0], np.cumsum(counts)])
    TPB = max(1, int(np.ceil(counts.max() / P)))
    NCH = int(math.ceil(TPB / CL))
    TPBp = NCH * CL
    EB = TPBp * P  # padded edges per block

    # per-edge histogram over bond vocabulary, scaled by 1/cnt
    hist = np.bincount(
        (np.arange(E, dtype=np.int64)[:, None] * BOND + s_attr).ravel(),
        minlength=E * BOND).reshape(E, BOND).astype(np.float32)
    cnt = np.maximum((s_attr != 0).sum(1), 1).astype(np.float32)
    hist_s = hist / cnt[:, None]

    idx_cols = NCH * (CL * P // 16)   # int16 index columns per block
    src_idx = np.zeros((CORES, P, BPC * idx_cols), np.int16)
    tgt_idx = np.zeros((CORES, P, BPC * idx_cols), np.int16)
    tgt_rel = np.full((CORES, P, BPC * TPBp), -1.0, np.float32)
    histT = np.zeros((CORES, BOND, BPC * TPBp * P), np.float32)
    deg_pad = np.ones(NPAD, np.float32)
    deg_pad[:N] = deg
    deg_blk = np.zeros((CORES, P, BPC), np.float32)

    for g in range(NSLOTS):
        c, i = divmod(g, BPC)
        e0, e1 = int(starts[g]), int(starts[g + 1])
        ne = e1 - e0
        sv = np.zeros(EB, np.int64)
        tv = np.zeros(EB, np.int64)
        tr = np.full(EB, -1.0, np.float32)
        hs = np.zeros((EB, BOND), np.float32)
        sv[:ne] = s_src[e0:e1]
        tv[:ne] = s_tgt[e0:e1]
        tr[:ne] = (s_tgt[e0:e1] - g * P).astype(np.float32)
        hs[:ne] = hist_s[e0:e1]
        # gather index wrap: index j of a chunk lives at [j % 16, j // 16]
        si = sv.reshape(NCH, CL * P).astype(np.int16)
        ti = tv.reshape(NCH, CL * P).astype(np.int16)
        si = si.reshape(NCH, CL * P // 16, 16).transpose(2, 0, 1).reshape(16, idx_cols)
        ti = ti.reshape(NCH, CL * P // 16, 16).transpose(2, 0, 1).reshape(16, idx_cols)
        # firmware: rx core reads partitions 0-15, tx core 16-31 (per queue) —
        # indices must be replicated across all 16-partition groups
        src_idx[c, :, i * idx_cols:(i + 1) * idx_cols] = np.tile(si, (8, 1))
        tgt_idx[c, :, i * idx_cols:(i + 1) * idx_cols] = np.tile(ti, (8, 1))
        tgt_rel[c, :, i * TPBp:(i + 1) * TPBp] = tr.reshape(TPBp, P).T
        histT[c, :, i * TPBp * P:(i + 1) * TPBp * P] = (
            hs.reshape(TPBp, P, BOND).transpose(2, 0, 1).reshape(BOND, TPBp * P))
        deg_blk[c, :, i] = deg_pad[g * P:(g + 1) * P]

    xT = np.zeros((W, NPAD), np.float32)
    xT[:, :N] = np.asarray(x, np.float32).T
    Wcat = np.concatenate([np.asarray(W_src, np.float32).T,
                           np.asarray(W_tgt, np.float32).T], axis=1)  # [256,512]
    BDgT = np.zeros((W, W), np.float32)
    BDvT = np.zeros((W, W), np.float32)
    for h in range(H):
        BDgT[h * D:(h + 1) * D, h * D:(h + 1) * D] = np.asarray(W_gate, np.float32)[h].T
        BDvT[h * D:(h + 1) * D, h * D:(h + 1) * D] = np.asarray(W_val, np.float32)[h].T
    BDgvT = np.concatenate([BDgT, BDvT], axis=1)                     # [256,512]
    EBD = np.concatenate([np.asarray(emb, np.float32) @ BDgT,
                          np.zeros((BOND, W), np.float32)], axis=1)  # [33,512]
    WpT = np.ascontiguousarray(np.asarray(W_post, np.float32).T)     # [256,256]
    p_bc = np.tile(np.asarray(deg_param, np.float32)[None, :], (P, 1))
    iota = np.tile(np.arange(P, dtype=np.float32)[None, :], (P, 1))

    shared = dict(xT=xT, Wcat=Wcat, BDgvT=BDgvT, EBD=EBD, WpT=WpT,
                  p_bc=p_bc, iota=iota)
    per_core = dict(src_idx=src_idx, tgt_idx=tgt_idx, tgt_rel=tgt_rel,
                    histT=histT, deg_blk=deg_blk)
    dims = dict(TPBp=TPBp, NCH=NCH, idx_cols=idx_cols)
    return shared, per_core, dims


def _build_program(dims):
    TPBp, NCH, idx_cols = dims["TPBp"], dims["NCH"], dims["idx_cols"]
    nc = bacc.Bacc("TRN2", target_bir_lowering=False, debug=False,
                   enable_asserts=False, num_devices=CORES)
    dt = {}
    def din(name, shape, dtype=FP):
        dt[name] = nc.dram_tensor(name, list(shape), dtype, kind="ExternalInput").ap()
        return dt[name]

    xT_d = din("xT", (W, NPAD))
    Wcat_d = din("Wcat", (W, 512))
    BDgvT_d = din("BDgvT", (W, 512))
    EBD_d = din("EBD", (BOND, 512))
    WpT_d = din("WpT", (W, W))
    p_bc_d = din("p_bc", (P, W))
    iota_d = din("iota", (P, P))
    srci_d = din("src_idx", (P, BPC * idx_cols), I16)
    tgti_d = din("tgt_idx", (P, BPC * idx_cols), I16)
    tgtr_d = din("tgt_rel", (P, BPC * TPBp))
    histT_d = din("histT", (BOND, BPC * TPBp * P))
    degb_d = din("deg_blk", (P, BPC))
    out_d = nc.dram_tensor("out", [P, BPC * W], FP, kind="ExternalOutput").ap()
    xst_d = nc.dram_tensor("xst_int", [NPAD, 2 * W], FP, kind="Internal").ap()
    dbg = {}
    if DEBUG:
        for nm, shp in [("d_xsg", (P, W)), ("d_xtg", (P, W)), ("d_xx", (P, W)),
                        ("d_xn", (P, W)), ("d_xnT", (P, 2 * P)),
                        ("d_gv", (P, 512)), ("d_act", (P, W)),
                        ("d_st", (P, P)), ("d_agg", (P, W)),
                        ("d_xst", (P, 2 * W))]:
            dbg[nm] = nc.dram_tensor(nm, list(shp), FP, kind="ExternalOutput").ap()

    add, mult, sub = (mybir.AluOpType.add, mybir.AluOpType.mult,
                      mybir.AluOpType.subtract)
    is_eq, amax = mybir.AluOpType.is_equal, mybir.AluOpType.max
    AF = mybir.ActivationFunctionType

    with tile.TileContext(nc) as tc:
        # ---------------- constants ----------------
        with tc.tile_pool(name="const", bufs=1) as cpool:
            ident = cpool.tile([P, P], FP)
            make_identity(nc, ident[:])
            eps_t = cpool.tile([P, 1], FP)
            nc.gpsimd.memset(eps_t[:], EPS_GN)
            iota_s = cpool.tile([P, P], FP)
            nc.sync.dma_start(iota_s[:], iota_d)
            wcat_s = cpool.tile([P, 2, 512], FP)
            nc.sync.dma_start(wcat_s[:], Wcat_d.rearrange("(k p) n -> p k n", p=P))
            bdgv_s = cpool.tile([P, 2, 512], FP)
            nc.sync.dma_start(bdgv_s[:], BDgvT_d.rearrange("(k p) n -> p k n", p=P))
            ebd_s = cpool.tile([BOND, 512], FP)
            nc.sync.dma_start(ebd_s[:], EBD_d)
            wpt_s = cpool.tile([P, 2, W], FP)
            nc.sync.dma_start(wpt_s[:], WpT_d.rearrange("(k p) n -> p k n", p=P))
            pbc_s = cpool.tile([P, W], FP)
            nc.sync.dma_start(pbc_s[:], p_bc_d)
            degb_s = cpool.tile([P, BPC], FP)
            nc.sync.dma_start(degb_s[:], degb_d)

            # ---------------- phase A: xs|xt tables ----------------
            with tc.tile_pool(name="pha", bufs=2) as apool, \
                 tc.tile_pool(name="pha_ps", bufs=2, space="PSUM") as apsum:
                xTs = apool.tile([P, 2, NPAD], FP, tag="xT")
                nc.sync.dma_start(xTs[:], xT_d.rearrange("(k p) n -> p k n", p=P))
                for nt in range(NSLOTS):
                    ps = apsum.tile([P, 512], FP)
                    for k in range(2):
                        nc.tensor.matmul(ps[:], lhsT=xTs[:, k, nt * P:(nt + 1) * P],
                                         rhs=wcat_s[:, k, :],
                                         start=(k == 0), stop=(k == 1))
                    sb = apool.tile([P, 512], FP, tag="xstcp")
                    if nt % 2 == 0:
                        nc.vector.tensor_copy(sb[:], ps[:])
                    else:
                        nc.scalar.copy(sb[:], ps[:])
                    nc.sync.dma_start(xst_d[nt * P:(nt + 1) * P, :], sb[:])

            # ---------------- phase B ----------------
            with tc.tile_pool(name="gat", bufs=2) as gpool, \
                 tc.tile_pool(name="blkin", bufs=2) as bpool, \
                 tc.tile_pool(name="work", bufs=3) as wpool, \
                 tc.tile_pool(name="small", bufs=3) as spool, \
                 tc.tile_pool(name="ps_gv", bufs=2, space="PSUM") as ps_gv, \
                 tc.tile_pool(name="ps_tr", bufs=2, space="PSUM") as ps_tr, \
                 tc.tile_pool(name="ps_agg", bufs=2, space="PSUM") as ps_agg:
                for i in range(BPC):
                    sidx = bpool.tile([P, idx_cols], I16, tag="sidx")
                    nc.sync.dma_start(sidx[:], srci_d[:, i * idx_cols:(i + 1) * idx_cols])
                    tidx = bpool.tile([P, idx_cols], I16, tag="tidx")
                    nc.sync.dma_start(tidx[:], tgti_d[:, i * idx_cols:(i + 1) * idx_cols])
                    trel = bpool.tile([P, TPBp], FP, tag="trel")
                    nc.sync.dma_start(trel[:], tgtr_d[:, i * TPBp:(i + 1) * TPBp])

                    agg = ps_agg.tile([P, W], FP)
                    for ch in range(NCH):
                        ccols = CL * P // 16
                        xs_g = gpool.tile([P, CL, W], FP, tag="xs_g")
                        nc.gpsimd.dma_gather(
                            out_ap=xs_g[:], in_ap=xst_d[:, 0:W],
                            idxs_ap=sidx[:, ch * ccols:(ch + 1) * ccols],
                            num_idxs=CL * P, num_idxs_reg=CL * P,
                            elem_size=W, elem_step=2 * W)
                        xt_g = gpool.tile([P, CL, W], FP, tag="xt_g")
                        nc.gpsimd.dma_gather(
                            out_ap=xt_g[:], in_ap=xst_d[:, W:2 * W],
                            idxs_ap=tidx[:, ch * ccols:(ch + 1) * ccols],
                            num_idxs=CL * P, num_idxs_reg=CL * P,
                            elem_size=W, elem_step=2 * W)
                        hT = bpool.tile([BOND, CL * P], FP, tag="hT")
                        nc.sync.dma_start(
                            hT[:], histT_d[:, (i * TPBp + ch * CL) * P:
                                           (i * TPBp + (ch + 1) * CL) * P])
                        for t_in in range(CL):
                            t = ch * CL + t_in
                            xx = wpool.tile([P, W], FP, tag="xx")
                            nc.gpsimd.tensor_tensor(xx[:], xs_g[:, t_in, :],
                                                    xt_g[:, t_in, :], add)
                            xxg = xx[:].rearrange("p (g d) -> p g d", g=H)
                            ssum = spool.tile([P, H], FP, tag="ssum")
                            nc.vector.reduce_sum(ssum[:], xxg, axis=mybir.AxisListType.X)
                            sq = wpool.tile([P, W], FP, tag="sq")
                            nc.scalar.square(sq[:], xx[:])
                            ssq = spool.tile([P, H], FP, tag="ssq")
                            nc.vector.reduce_sum(
                                ssq[:], sq[:].rearrange("p (g d) -> p g d", g=H),
                                axis=mybir.AxisListType.X)
                            mun = spool.tile([P, H], FP, tag="mun")
                            nc.vector.tensor_scalar(mun[:], ssum[:], -1.0 / 32.0,
                                                    None, mult)
                            s2 = spool.tile([P, H], FP, tag="s2")
                            nc.vector.tensor_tensor(s2[:], ssum[:], ssum[:], mult)
                            m2t = spool.tile([P, H], FP, tag="m2t")
                            nc.vector.scalar_tensor_tensor(
                                m2t[:], s2[:], -1.0 / 32.0, ssq[:], mult, add)
                            std = spool.tile([P, H], FP, tag="std")
                            nc.scalar.activation(std[:], m2t[:], AF.Sqrt,
                                                 bias=eps_t[:], scale=1.0 / 32.0)
                            rsig = spool.tile([P, H], FP, tag="rsig")
                            nc.vector.reciprocal(rsig[:], std[:])
                            xn = wpool.tile([P, W], FP, tag="xn")
                            for h in range(H):
                                nc.vector.tensor_scalar(
                                    xn[:, h * D:(h + 1) * D],
                                    xx[:, h * D:(h + 1) * D],
                                    mun[:, h:h + 1], rsig[:, h:h + 1], add, mult)
                            xnT = wpool.tile([P, 2, P], FP, tag="xnT")
                            for k in range(2):
                                tp = ps_tr.tile([P, P], FP, tag="tp")
                                nc.tensor.transpose(tp[:], xn[:, k * P:(k + 1) * P],
                                                    ident[:])
                                nc.scalar.copy(xnT[:, k, :], tp[:])
                            gv = ps_gv.tile([P, 512], FP)
                            nc.tensor.matmul(gv[:], lhsT=xnT[:, 0, :],
                                             rhs=bdgv_s[:, 0, :], start=True, stop=False)
                            nc.tensor.matmul(gv[:], lhsT=xnT[:, 1, :],
                                             rhs=bdgv_s[:, 1, :], start=False, stop=False)
                            nc.tensor.matmul(gv[:], lhsT=hT[:, t_in * P:(t_in + 1) * P],
                                             rhs=ebd_s[:], start=False, stop=True)
                            gate = wpool.tile([P, W], FP, tag="gate")
                            nc.scalar.activation(gate[:], gv[:, 0:W], AF.Relu)
                            act = wpool.tile([P, W], FP, tag="act")
                            nc.vector.tensor_tensor(act[:], gate[:],
                                                    gv[:, W:2 * W], mult)
                            st = wpool.tile([P, P], FP, tag="st")
                            nc.vector.tensor_scalar(st[:], iota_s[:],
                                                    trel[:, t:t + 1], None, is_eq)
                            nc.tensor.matmul(agg[:], lhsT=st[:], rhs=act[:],
                                             start=(t == 0), stop=(t == TPBp - 1),
                                             skip_group_check=True)
                            if DEBUG and i == 0 and t == 0:
                                for nm, tl in [("d_xsg", xs_g[:, 0, :]),
                                               ("d_xtg", xt_g[:, 0, :]),
                                               ("d_xx", xx[:]), ("d_xn", xn[:]),
                                               ("d_xnT", xnT[:]), ("d_gv", gv[:]),
                                               ("d_act", act[:]), ("d_st", st[:])]:
                                    if tl.tensor.space == bass.MemorySpace.PSUM:
                                        cp = wpool.tile(list(tl.shape), FP, tag="dbgcp")
                                        nc.scalar.copy(cp[:], tl)
                                        tl = cp[:]
                                    nc.sync.dma_start(dbg[nm], tl)
                    # ---- block epilogue ----
                    aggs = wpool.tile([P, W], FP, tag="aggs")
                    nc.scalar.copy(aggs[:], agg[:])
                    if DEBUG and i == 0:
                        nc.sync.dma_start(dbg["d_agg"], aggs[:])
                        xst_cp = wpool.tile([P, 2 * W], FP, tag="dbgxst")
                        nc.sync.dma_start(xst_cp[:], xst_d[0:P, :])
                        nc.sync.dma_start(dbg["d_xst"], xst_cp[:])
                    aggT = wpool.tile([P, 2, P], FP, tag="aggT")
                    for k in range(2):
                        tp = ps_tr.tile([P, P], FP, tag="tp")
                        nc.tensor.transpose(tp[:], aggs[:, k * P:(k + 1) * P], ident[:])
                        nc.scalar.copy(aggT[:, k, :], tp[:])
                    ops = ps_gv.tile([P, 512], FP)
                    for k in range(2):
                        nc.tensor.matmul(ops[:, 0:W], lhsT=aggT[:, k, :],
                                         rhs=wpt_s[:, k, :],
                                         start=(k == 0), stop=(k == 1))
                    logd = spool.tile([P, 1], FP, tag="logd")
                    nc.scalar.activation(logd[:], degb_s[:, i:i + 1], AF.Ln)
                    tsc = wpool.tile([P, W], FP, tag="tsc")
                    nc.vector.tensor_scalar(tsc[:], pbc_s[:], logd[:], None, mult)
                    sc = wpool.tile([P, W], FP, tag="sc")
                    nc.scalar.activation(sc[:], tsc[:], AF.Exp)
                    outf = wpool.tile([P, W], FP, tag="outf")
                    nc.vector.tensor_tensor(outf[:], sc[:], ops[:, 0:W], mult)
                    nc.sync.dma_start(out_d[:, i * W:(i + 1) * W], outf[:])
    nc.compile()
    return nc


def _assemble(results):
    out = np.empty((NPAD, W), np.float32)
    for c in range(CORES):
        oc = results[c]["out"]  # [P, BPC*W]
        out[c * BPC * P:(c + 1) * BPC * P] = (
            oc.reshape(P, BPC, W).transpose(1, 0, 2).reshape(BPC * P, W))
    return out[:N]


def prepare(inputs):
    shared, per_core, dims = _prep_host(**inputs)
    nc = _build_program(dims)
    in_maps = []
    for c in range(CORES):
        m = dict(shared)
        for k, v in per_core.items():
            m[k] = np.ascontiguousarray(v[c])
        in_maps.append(m)
    return nc, in_maps


def run_pjrt_timed(nc, in_maps, iters=0):
    """Execute the program on the 8 cores via PJRT (axon). Returns
    (per-core result dicts, per-iteration wall ns or None).

    Mirrors bass2jax.run_bass_via_pjrt's multi-core path, but keeps the
    jitted callable so repeated executions can be timed with inputs
    already resident on device."""
    import time

    import jax
    from jax.sharding import Mesh, NamedSharding, PartitionSpec
    from jax.experimental.shard_map import shard_map

    from concourse import bass2jax, mybir as mb
    bass2jax.install_neuronx_cc_hook()

    n_cores = CORES
    partition_name = (nc.partition_id_tensor.name
                      if nc.partition_id_tensor else None)
    in_names, out_names, out_avals, zero_outs = [], [], [], []
    for alloc in nc.m.functions[0].allocations:
        if not isinstance(alloc, mb.MemoryLocationSet):
            continue
        name = alloc.memorylocations[0].name
        if alloc.kind == "ExternalInput":
            if name != partition_name:
                in_names.append(name)
        elif alloc.kind == "ExternalOutput":
            shape = tuple(alloc.tensor_shape)
            dtype = mb.dt.np(alloc.dtype)
            out_names.append(name)
            out_avals.append(jax.core.ShapedArray(shape, dtype))
            zero_outs.append(np.zeros(shape, dtype))
    n_params = len(in_names)
    n_outs = len(out_avals)
    in_names.extend(out_names)
    if partition_name is not None:
        in_names.append(partition_name)
    donate = tuple(range(n_params, n_params + n_outs))

    def _body(*args):
        operands = list(args)
        if partition_name is not None:
            operands.append(bass2jax.partition_id_tensor())
        outs = bass2jax._bass_exec_p.bind(
            *operands, out_avals=tuple(out_avals), in_names=tuple(in_names),
            out_names=tuple(out_names), lowering_input_output_aliases=(),
            sim_require_finite=True, sim_require_nnan=True, nc=nc)
        return tuple(outs)

    devices = jax.devices()[:n_cores]
    mesh = Mesh(np.asarray(devices), ("core",))
    sharded = jax.jit(
        shard_map(_body, mesh=mesh,
                  in_specs=(PartitionSpec("core"),) * (n_params + n_outs),
                  out_specs=(PartitionSpec("core"),) * len(out_names),
                  check_rep=False),
        donate_argnums=donate, keep_unused=True)

    sh = NamedSharding(mesh, PartitionSpec("core"))
    concat_in = [
        jax.device_put(
            np.concatenate([np.asarray(in_maps[c][nm]) for c in range(n_cores)],
                           axis=0), sh)
        for nm in in_names[:n_params]]
    def zeros_dev():
        return [jax.device_put(
            np.zeros((n_cores * z.shape[0], *z.shape[1:]), z.dtype), sh)
            for z in zero_outs]

    out_arrs = jax.block_until_ready(sharded(*concat_in, *zeros_dev()))
    results = [
        {nm: np.asarray(out_arrs[i]).reshape(n_cores, *out_avals[i].shape)[c]
         for i, nm in enumerate(out_names)}
        for c in range(n_cores)]

    per_iter_ns = None
    if iters > 0:
        zsets = [zeros_dev() for _ in range(iters + 2)]
        jax.block_until_ready(zsets)
        # warmup 2
        jax.block_until_ready([sharded(*concat_in, *zsets[k]) for k in range(2)])
        t0 = time.perf_counter()
        outs = [sharded(*concat_in, *zsets[2 + k]) for k in range(iters)]
        jax.block_until_ready(outs)
        t1 = time.perf_counter()
        per_iter_ns = (t1 - t0) / iters * 1e9
    return results, per_iter_ns


def _run(inputs, iters=0):
    nc, in_maps = prepare(inputs)
    results, per_iter_ns = run_pjrt_timed(nc, in_maps, iters=iters)
    return _assemble(results), per_iter_ns


def kernel(**inputs) -> np.ndarray:
    out, _ = _run(inputs, iters=0)
    return out


# revision 21
# speedup vs baseline: 1.1394x; 1.1394x over previous
"""Trainium2 Bass kernel for MetaGIN-style GNN message passing.

Strategy (edge-sharded, tgt-sorted):
  * Sort edges by target node; group nodes into 128-node blocks; assign
    blocks to 8 cores x BPC slots, bucketed by edge count so that every
    core's slot i has a similar tile count (the per-slot tile counts are
    part of the compiled program and must match across cores).
  * Phase A (replicated): xs|xt = x @ [W_src.T | W_tgt.T] -> internal DRAM.
  * Phase B per block: dma_gather xs[src]/xt[tgt] rows, GroupNorm
    (fused sum|sumsq reduce + broadcast-AP normalize), gated MLP via
    bf16 block-diagonal matmuls (EmbeddingBag folded in as a
    host-precomputed histogram matmul accumulated in the same PSUM
    group), scatter-add to nodes via a one-hot bf16 matmul accumulating
    in PSUM across the block's edge tiles.
  * Per block: agg @ W_post.T (f32), deg^deg_param scaling, write out.
  No collectives: each core owns a disjoint set of 128-node blocks.
"""

import math

import ml_dtypes
import numpy as np

import concourse.bass as bass
import concourse.mybir as mybir
import concourse.tile as tile
from concourse import bacc
from concourse.masks import make_identity

P = 128
N = 10000
W = 256
H = 8
D = 32
BOND = 33
E = 320000
EPS_GN = 1e-5
CORES = 8
NPAD = 10240          # 80 * 128
NSLOTS = NPAD // P    # 80 block slots
BPC = NSLOTS // CORES  # 10 block slots per core
CL = 8                # max tiles (of 128 edges) per gather chunk
FP = mybir.dt.float32
BF = mybir.dt.bfloat16
I16 = mybir.dt.int16
BF_NP = ml_dtypes.bfloat16
DEBUG = False  # adds intermediate-dump outputs for core 0 / block 0 / tile 0


def _prep_host(x, deg, edge_idx, edge_attr, W_src, W_tgt, emb, W_gate, W_val,
               W_post, deg_param):
    src = np.ascontiguousarray(edge_idx[0]).astype(np.int64)
    tgt = np.ascontiguousarray(edge_idx[1]).astype(np.int64)
    attr = np.asarray(edge_attr).astype(np.int64)

    order = np.argsort(tgt, kind="stable")
    s_src = src[order]
    s_tgt = tgt[order]
    s_attr = attr[order]

    blk = (s_tgt // P).astype(np.int64)
    counts = np.bincount(blk, minlength=NSLOTS)
    starts = np.concatenate([[0], np.cumsum(counts)])

    # bucket blocks by descending edge count: slot i takes ranks [8i, 8i+8)
    rank = np.argsort(-counts, kind="stable")
    blk_of = np.empty((CORES, BPC), np.int64)
    TPB = np.empty(BPC, np.int64)  # tiles per slot (same across cores)
    for i in range(BPC):
        grp = rank[i * CORES:(i + 1) * CORES]
        for c in range(CORES):
            blk_of[c, i] = grp[c]
        TPB[i] = max(1, int(math.ceil(counts[grp].max() / P)))
    TOFF = np.concatenate([[0], np.cumsum(TPB)])  # tile offsets per slot
    TT = int(TOFF[-1])                            # total tiles per core
    # gather chunks per slot: full CL-tile chunks + remainder
    CHS = []  # list of lists of chunk tile-counts
    for i in range(BPC):
        nfull, rem = divmod(int(TPB[i]), CL)
        CHS.append([CL] * nfull + ([rem] if rem else []))

    # per-edge histogram over bond vocabulary, scaled by 1/cnt
    hist = np.bincount(
        (np.arange(E, dtype=np.int64)[:, None] * BOND + s_attr).ravel(),
        minlength=E * BOND).reshape(E, BOND).astype(np.float32)
    cnt = np.maximum((s_attr != 0).sum(1), 1).astype(np.float32)
    hist_s = hist / cnt[:, None]

    src_idx = np.zeros((CORES, P, TT * 8), np.int16)
    tgt_idx = np.zeros((CORES, P, TT * 8), np.int16)
    tgt_rel = np.full((CORES, P, TT), -1.0, np.float32)
    histT = np.zeros((CORES, BOND, TT * P), BF_NP)
    deg_pad = np.ones(NPAD, np.float32)
    deg_pad[:N] = deg
    deg_blk = np.zeros((CORES, P, BPC), np.float32)

    for c in range(CORES):
        for i in range(BPC):
            g = int(blk_of[c, i])
            e0, e1 = int(starts[g]), int(starts[g + 1])
            ne = e1 - e0
            EB = int(TPB[i]) * P
            sv = np.zeros(EB, np.int64)
            tv = np.zeros(EB, np.int64)
            tr = np.full(EB, -1.0, np.float32)
            hs = np.zeros((EB, BOND), np.float32)
            sv[:ne] = s_src[e0:e1]
            tv[:ne] = s_tgt[e0:e1]
            tr[:ne] = (s_tgt[e0:e1] - g * P).astype(np.float32)
            hs[:ne] = hist_s[e0:e1]
            # idx wrap: flat index j of a chunk lives at [j % 16, j // 16],
            # replicated across all 16-partition groups (rx+tx Q7 cores)
            icols = int(TPB[i]) * 8
            si = sv.astype(np.int16).reshape(icols, 16).T
            ti = tv.astype(np.int16).reshape(icols, 16).T
            o8 = int(TOFF[i]) * 8
            src_idx[c, :, o8:o8 + icols] = np.tile(si, (8, 1))
            tgt_idx[c, :, o8:o8 + icols] = np.tile(ti, (8, 1))
            ot = int(TOFF[i])
            tgt_rel[c, :, ot:ot + int(TPB[i])] = tr.reshape(-1, P).T
            histT[c, :, ot * P:(ot + int(TPB[i])) * P] = (
                hs.reshape(-1, P, BOND).transpose(2, 0, 1)
                .reshape(BOND, EB).astype(BF_NP))
            deg_blk[c, :, i] = deg_pad[g * P:(g + 1) * P]

    xT = np.zeros((W, NPAD), BF_NP)
    xT[:, :N] = np.asarray(x, np.float32).T.astype(BF_NP)
    Wcat = np.concatenate([np.asarray(W_src, np.float32).T,
                           np.asarray(W_tgt, np.float32).T],
                          axis=1).astype(BF_NP)  # [256,512]
    BDgT = np.zeros((W, W), np.float32)
    BDvT = np.zeros((W, W), np.float32)
    for h in range(H):
        BDgT[h * D:(h + 1) * D, h * D:(h + 1) * D] = np.asarray(W_gate, np.float32)[h].T
        BDvT[h * D:(h + 1) * D, h * D:(h + 1) * D] = np.asarray(W_val, np.float32)[h].T
    BDgvT = np.concatenate([BDgT, BDvT], axis=1).astype(BF_NP)       # [256,512]
    EBD = np.concatenate([np.asarray(emb, np.float32) @ BDgT,
                          np.zeros((BOND, W), np.float32)],
                         axis=1).astype(BF_NP)                       # [33,512]
    WpT = np.ascontiguousarray(np.asarray(W_post, np.float32).T)     # [256,256]
    p_bc = np.tile(np.asarray(deg_param, np.float32)[None, :], (P, 1))
    iota = np.tile(np.arange(P, dtype=np.float32)[None, :], (P, 1))

    shared = dict(xT=xT, Wcat=Wcat, BDgvT=BDgvT, EBD=EBD, WpT=WpT,
                  p_bc=p_bc, iota=iota)
    per_core = dict(src_idx=src_idx, tgt_idx=tgt_idx, tgt_rel=tgt_rel,
                    histT=histT, deg_blk=deg_blk)
    dims = dict(TPB=[int(v) for v in TPB], TOFF=[int(v) for v in TOFF],
                TT=TT, CHS=CHS, blk_of=blk_of)
    return shared, per_core, dims


def _build_program(dims):
    TPB, TOFF, TT, CHS = dims["TPB"], dims["TOFF"], dims["TT"], dims["CHS"]
    nc = bacc.Bacc("TRN2", target_bir_lowering=False, debug=False,
                   enable_asserts=False, num_devices=CORES)

    def din(name, shape, dtype=FP):
        return nc.dram_tensor(name, list(shape), dtype, kind="ExternalInput").ap()

    xT_d = din("xT", (W, NPAD), BF)
    Wcat_d = din("Wcat", (W, 512), BF)
    BDgvT_d = din("BDgvT", (W, 512), BF)
    EBD_d = din("EBD", (BOND, 512), BF)
    WpT_d = din("WpT", (W, W))
    p_bc_d = din("p_bc", (P, W))
    iota_d = din("iota", (P, P))
    srci_d = din("src_idx", (P, TT * 8), I16)
    tgti_d = din("tgt_idx", (P, TT * 8), I16)
    tgtr_d = din("tgt_rel", (P, TT))
    histT_d = din("histT", (BOND, TT * P), BF)
    degb_d = din("deg_blk", (P, BPC))
    out_d = nc.dram_tensor("out", [P, BPC * W], FP, kind="ExternalOutput").ap()
    xst_d = nc.dram_tensor("xst_int", [NPAD, 2 * W], BF, kind="Internal").ap()
    dbg = {}
    if DEBUG:
        for nm, shp in [("d_xx", (P, W)), ("d_xn", (P, W)),
                        ("d_gv", (P, 512)), ("d_act", (P, W)),
                        ("d_agg", (P, W))]:
            dbg[nm] = nc.dram_tensor(nm, list(shp), FP, kind="ExternalOutput").ap()

    add, mult = mybir.AluOpType.add, mybir.AluOpType.mult
    is_eq = mybir.AluOpType.is_equal
    AF = mybir.ActivationFunctionType

    with tile.TileContext(nc) as tc:
        # ---------------- constants ----------------
        with tc.tile_pool(name="const", bufs=1) as cpool:
            ident = cpool.tile([P, P], BF)
            make_identity(nc, ident[:])
            identf = cpool.tile([P, P], FP)
            make_identity(nc, identf[:])
            eps_t = cpool.tile([P, 1], FP)
            nc.gpsimd.memset(eps_t[:], EPS_GN)
            iota_s = cpool.tile([P, P], FP)
            nc.sync.dma_start(iota_s[:], iota_d)
            wcat_s = cpool.tile([P, 2, 512], BF)
            nc.sync.dma_start(wcat_s[:], Wcat_d.rearrange("(k p) n -> p k n", p=P))
            bdgv_s = cpool.tile([P, 2, 512], BF)
            nc.sync.dma_start(bdgv_s[:], BDgvT_d.rearrange("(k p) n -> p k n", p=P))
            ebd_s = cpool.tile([BOND, 512], BF)
            nc.sync.dma_start(ebd_s[:], EBD_d)
            wpt_s = cpool.tile([P, 2, W], FP)
            nc.sync.dma_start(wpt_s[:], WpT_d.rearrange("(k p) n -> p k n", p=P))
            pbc_s = cpool.tile([P, W], FP)
            nc.sync.dma_start(pbc_s[:], p_bc_d)
            degb_s = cpool.tile([P, BPC], FP)
            nc.sync.dma_start(degb_s[:], degb_d)

            # ---------------- phase A: xs|xt tables ----------------
            with tc.tile_pool(name="pha", bufs=2) as apool, \
                 tc.tile_pool(name="pha_ps", bufs=2, space="PSUM") as apsum:
                xTs = apool.tile([P, 2, NPAD], BF, tag="xT")
                nc.sync.dma_start(xTs[:], xT_d.rearrange("(k p) n -> p k n", p=P))
                for nt in range(NSLOTS):
                    ps = apsum.tile([P, 512], FP)
                    for k in range(2):
                        nc.tensor.matmul(ps[:], lhsT=xTs[:, k, nt * P:(nt + 1) * P],
                                         rhs=wcat_s[:, k, :],
                                         start=(k == 0), stop=(k == 1))
                    sb = apool.tile([P, 512], BF, tag="xstcp")
                    if nt % 2 == 0:
                        nc.vector.tensor_copy(sb[:], ps[:])
                    else:
                        nc.scalar.copy(sb[:], ps[:])
                    nc.sync.dma_start(xst_d[nt * P:(nt + 1) * P, :], sb[:])

            # ---------------- phase B ----------------
            with tc.tile_pool(name="gat", bufs=3) as gpool, \
                 tc.tile_pool(name="blkin", bufs=2) as bpool, \
                 tc.tile_pool(name="work", bufs=8) as wpool, \
                 tc.tile_pool(name="small", bufs=10) as spool, \
                 tc.tile_pool(name="ps_gv", bufs=3, space="PSUM") as ps_gv, \
                 tc.tile_pool(name="ps_tr", bufs=2, space="PSUM") as ps_tr, \
                 tc.tile_pool(name="ps_agg", bufs=2, space="PSUM") as ps_agg:
                for i in range(BPC):
                    tpb, toff = TPB[i], TOFF[i]
                    icols = tpb * 8
                    sidx = bpool.tile([P, icols], I16, tag="sidx")
                    nc.sync.dma_start(sidx[:], srci_d[:, toff * 8:toff * 8 + icols])
                    tidx = bpool.tile([P, icols], I16, tag="tidx")
                    nc.sync.dma_start(tidx[:], tgti_d[:, toff * 8:toff * 8 + icols])
                    trel = bpool.tile([P, tpb], FP, tag="trel")
                    nc.sync.dma_start(trel[:], tgtr_d[:, toff:toff + tpb])

                    agg = ps_agg.tile([P, W], FP)
                    t = 0
                    for clen in CHS[i]:
                        ccols = clen * 8
                        ni = clen * P
                        xs_g = gpool.tile([P, CL, W], BF, tag="xs_g")
                        nc.gpsimd.dma_gather(
                            out_ap=xs_g[:, 0:clen, :], in_ap=xst_d[:, 0:W],
                            idxs_ap=sidx[:, t * 8:t * 8 + ccols],
                            num_idxs=ni, num_idxs_reg=ni,
                            elem_size=W, elem_step=2 * W)
                        xt_g = gpool.tile([P, CL, W], BF, tag="xt_g")
                        nc.gpsimd.dma_gather(
                            out_ap=xt_g[:, 0:clen, :], in_ap=xst_d[:, W:2 * W],
                            idxs_ap=tidx[:, t * 8:t * 8 + ccols],
                            num_idxs=ni, num_idxs_reg=ni,
                            elem_size=W, elem_step=2 * W)
                        hT = bpool.tile([BOND, CL * P], BF, tag="hT")
                        nc.sync.dma_start(
                            hT[:, 0:ni],
                            histT_d[:, (toff + t) * P:(toff + t + clen) * P])
                        xns = []
                        xxsqs, statss, muns, s2s, m2ts, stds, rsigs, xn32s = \
                            [], [], [], [], [], [], [], []
                        for t_in in range(clen):
                            xxsq = wpool.tile([P, 2, W], FP, tag="xxsq")
                            nc.gpsimd.tensor_tensor(xxsq[:, 0, :], xs_g[:, t_in, :],
                                                    xt_g[:, t_in, :], add)
                            xxsqs.append(xxsq)
                        for t_in in range(clen):
                            nc.scalar.square(xxsqs[t_in][:, 1, :],
                                             xxsqs[t_in][:, 0, :])
                        for t_in in range(clen):
                            stats = spool.tile([P, 2, H], FP, tag="stats")
                            nc.vector.reduce_sum(
                                stats[:], xxsqs[t_in][:].rearrange(
                                    "p b (g d) -> p b g d", g=H),
                                axis=mybir.AxisListType.X)
                            statss.append(stats)
                        for t_in in range(clen):
                            ssum = statss[t_in][:, 0, :]
                            mun = spool.tile([P, H], FP, tag="mun")
                            nc.gpsimd.tensor_scalar(mun[:], ssum, -1.0 / 32.0,
                                                    None, mult)
                            muns.append(mun)
                            s2 = spool.tile([P, H], FP, tag="s2")
                            nc.gpsimd.tensor_tensor(s2[:], ssum, ssum, mult)
                            s2s.append(s2)
                        for t_in in range(clen):
                            m2t = spool.tile([P, H], FP, tag="m2t")
                            nc.vector.scalar_tensor_tensor(
                                m2t[:], s2s[t_in][:], -1.0 / 32.0,
                                statss[t_in][:, 1, :], mult, add)
                            m2ts.append(m2t)
                        for t_in in range(clen):
                            std = spool.tile([P, H], FP, tag="std")
                            nc.scalar.activation(std[:], m2ts[t_in][:], AF.Sqrt,
                                                 bias=eps_t[:], scale=1.0 / 32.0)
                            stds.append(std)
                        for t_in in range(clen):
                            rsig = spool.tile([P, H], FP, tag="rsig")
                            nc.vector.reciprocal(rsig[:], stds[t_in][:])
                            rsigs.append(rsig)
                        for t_in in range(clen):
                            xn32 = wpool.tile([P, H, D], FP, tag="xn32")
                            nc.vector.tensor_tensor(
                                xn32[:], xxsqs[t_in][:, 0, :].rearrange(
                                    "p (g d) -> p g d", g=H),
                                muns[t_in][:].to_broadcast((P, H, D)), add)
                            xn32s.append(xn32)
                        for t_in in range(clen):
                            xn = wpool.tile([P, H, D], BF, tag="xn")
                            nc.vector.tensor_tensor(
                                xn[:], xn32s[t_in][:],
                                rsigs[t_in][:].to_broadcast((P, H, D)), mult)
                            xns.append(xn)
                        for t_in in range(clen):
                            xn = xns[t_in]
                            xnf = xn[:].rearrange("p g d -> p (g d)")
                            xnT = wpool.tile([P, 2, P], BF, tag="xnT")
                            tp = ps_tr.tile([P, 2, P], BF, tag="tp")
                            for k in range(2):
                                nc.tensor.transpose(
                                    tp[:, k, :], xnf[:, k * P:(k + 1) * P],
                                    ident[:])
                            nc.scalar.copy(xnT[:], tp[:])
                            gv = ps_gv.tile([P, 512], FP)
                            nc.tensor.matmul(gv[:], lhsT=xnT[:, 0, :],
                                             rhs=bdgv_s[:, 0, :],
                                             start=True, stop=False)
                            nc.tensor.matmul(gv[:], lhsT=xnT[:, 1, :],
                                             rhs=bdgv_s[:, 1, :],
                                             start=False, stop=False)
                            nc.tensor.matmul(gv[:],
                                             lhsT=hT[:, t_in * P:(t_in + 1) * P],
                                             rhs=ebd_s[:], start=False, stop=True)
                            gate = wpool.tile([P, W], FP, tag="gate")
                            nc.scalar.activation(gate[:], gv[:, 0:W], AF.Relu)
                            act = wpool.tile([P, W], BF, tag="act")
                            nc.vector.tensor_tensor(act[:], gate[:],
                                                    gv[:, W:2 * W], mult)
                            st = wpool.tile([P, P], BF, tag="st")
                            tg = t + t_in
                            nc.gpsimd.tensor_scalar(st[:], iota_s[:],
                                                    trel[:, tg:tg + 1], None, is_eq)
                            nc.tensor.matmul(agg[:], lhsT=st[:], rhs=act[:],
                                             start=(tg == 0),
                                             stop=(tg == tpb - 1),
                                             skip_group_check=True)
                        t += clen
                    # ---- block epilogue ----
                    aggs = wpool.tile([P, W], FP, tag="aggs")
                    nc.scalar.copy(aggs[:], agg[:])
                    if DEBUG and i == 0:
                        nc.sync.dma_start(dbg["d_agg"], aggs[:])
                    aggT = wpool.tile([P, 2, P], FP, tag="aggT")
                    for k in range(2):
                        tpf = ps_tr.tile([P, P], FP, tag="tp")
                        nc.tensor.transpose(tpf[:], aggs[:, k * P:(k + 1) * P],
                                            identf[:])
                        nc.scalar.copy(aggT[:, k, :], tpf[:])
                    ops = ps_gv.tile([P, 512], FP, tag="gv")
                    for k in range(2):
                        nc.tensor.matmul(ops[:, 0:W], lhsT=aggT[:, k, :],
                                         rhs=wpt_s[:, k, :],
                                         start=(k == 0), stop=(k == 1))
                    logd = spool.tile([P, 1], FP, tag="logd")
                    nc.scalar.activation(logd[:], degb_s[:, i:i + 1], AF.Ln)
                    tsc = wpool.tile([P, W], FP, tag="tsc")
                    nc.vector.tensor_scalar(tsc[:], pbc_s[:], logd[:], None, mult)
                    sc = wpool.tile([P, W], FP, tag="sc")
                    nc.scalar.activation(sc[:], tsc[:], AF.Exp)
                    outf = wpool.tile([P, W], FP, tag="outf")
                    nc.vector.tensor_tensor(outf[:], sc[:], ops[:, 0:W], mult)
                    nc.sync.dma_start(out_d[:, i * W:(i + 1) * W], outf[:])
    nc.compile()
    return nc


def _assemble(results, dims):
    blk_of = dims["blk_of"]
    out = np.zeros((NPAD, W), np.float32)
    for c in range(CORES):
        oc = results[c]["out"].reshape(P, BPC, W)
        for i in range(BPC):
            g = int(blk_of[c][i]) if not isinstance(blk_of, np.ndarray) \
                else int(blk_of[c, i])
            out[g * P:(g + 1) * P] = oc[:, i, :]
    return out[:N]


def prepare(inputs):
    shared, per_core, dims = _prep_host(**inputs)
    nc = _build_program(dims)
    in_maps = []
    for c in range(CORES):
        m = dict(shared)
        for k, v in per_core.items():
            m[k] = np.ascontiguousarray(v[c])
        in_maps.append(m)
    return nc, in_maps, dims


def run_pjrt_timed(nc, in_maps, iters=0):
    """Execute on the 8 cores via PJRT (axon). Returns (per-core result
    dicts, per-iteration wall ns or None). Mirrors
    bass2jax.run_bass_via_pjrt's multi-core path but keeps the jitted
    callable so repeated executions are timed with device-resident inputs."""
    import time

    import jax
    from jax.experimental.shard_map import shard_map
    from jax.sharding import Mesh, NamedSharding, PartitionSpec

    from concourse import bass2jax, mybir as mb
    bass2jax.install_neuronx_cc_hook()

    n_cores = CORES
    partition_name = (nc.partition_id_tensor.name
                      if nc.partition_id_tensor else None)
    in_names, out_names, out_avals, zero_outs = [], [], [], []
    for alloc in nc.m.functions[0].allocations:
        if not isinstance(alloc, mb.MemoryLocationSet):
            continue
        name = alloc.memorylocations[0].name
        if alloc.kind == "ExternalInput":
            if name != partition_name:
                in_names.append(name)
        elif alloc.kind == "ExternalOutput":
            shape = tuple(alloc.tensor_shape)
            dtype = mb.dt.np(alloc.dtype)
            out_names.append(name)
            out_avals.append(jax.core.ShapedArray(shape, dtype))
            zero_outs.append(np.zeros(shape, dtype))
    n_params = len(in_names)
    n_outs = len(out_avals)
    in_names.extend(out_names)
    if partition_name is not None:
        in_names.append(partition_name)
    donate = tuple(range(n_params, n_params + n_outs))

    def _body(*args):
        operands = list(args)
        if partition_name is not None:
            operands.append(bass2jax.partition_id_tensor())
        outs = bass2jax._bass_exec_p.bind(
            *operands, out_avals=tuple(out_avals), in_names=tuple(in_names),
            out_names=tuple(out_names), lowering_input_output_aliases=(),
            sim_require_finite=True, sim_require_nnan=True, nc=nc)
        return tuple(outs)

    devices = jax.devices()[:n_cores]
    mesh = Mesh(np.asarray(devices), ("core",))
    sharded = jax.jit(
        shard_map(_body, mesh=mesh,
                  in_specs=(PartitionSpec("core"),) * (n_params + n_outs),
                  out_specs=(PartitionSpec("core"),) * len(out_names),
                  check_rep=False),
        donate_argnums=donate, keep_unused=True)

    sh = NamedSharding(mesh, PartitionSpec("core"))
    concat_in = [
        jax.device_put(
            np.concatenate([np.asarray(in_maps[c][nm]) for c in range(n_cores)],
                           axis=0), sh)
        for nm in in_names[:n_params]]

    def zeros_dev():
        return [jax.device_put(
            np.zeros((n_cores * z.shape[0], *z.shape[1:]), z.dtype), sh)
            for z in zero_outs]

    out_arrs = jax.block_until_ready(sharded(*concat_in, *zeros_dev()))
    results = [
        {nm: np.asarray(out_arrs[i]).reshape(n_cores, *out_avals[i].shape)[c]
         for i, nm in enumerate(out_names)}
        for c in range(n_cores)]

    per_iter_ns = None
    if iters > 0:
        zsets = [zeros_dev() for _ in range(iters + 2)]
        jax.block_until_ready(zsets)
        jax.block_until_ready([sharded(*concat_in, *zsets[k]) for k in range(2)])
        t0 = time.perf_counter()
        outs = [sharded(*concat_in, *zsets[2 + k]) for k in range(iters)]
        jax.block_until_ready(outs)
        t1 = time.perf_counter()
        per_iter_ns = (t1 - t0) / iters * 1e9
    return results, per_iter_ns


def _run(inputs, iters=0):
    nc, in_maps, dims = prepare(inputs)
    results, per_iter_ns = run_pjrt_timed(nc, in_maps, iters=iters)
    return _assemble(results, dims), per_iter_ns


def kernel(**inputs) -> np.ndarray:
    out, _ = _run(inputs, iters=0)
    return out
